# revision 3
# baseline (speedup 1.0000x reference)
"""Trainium2 Bass kernel for nn_DecoderBlock — sequence-parallel, zero collectives.

Each of 8 cores owns one batch element (r//4) and 512 query tokens (4
interleaved 128-blocks {q, 7-q, 8+q, 15-q} so causal work balances).
Every core computes full K/V (all heads, all 2048 keys) for itself —
replicated compute instead of cross-core communication.

All matmul operands are bf16 (full PE rate); PSUM accumulates fp32; the
residual stream lives in SBUF bf16. Causality is enforced by per-rank
mask tiles multiplied into exp(scores) — the program is rank-uniform,
only data differs. LayerNorm mean/var run on TensorE via ones-matmuls.

SBUF plan (KB/partition): const ~5, z 8, zln 8, p/pm 6, dn ~10,
attn-scope: k 64 + v 32.5 + q 8 + attn 8 + xq 4; phase pools stream x,
mem, and weights in 0.5-1K chunks per partition.
"""

import os
import sys

sys.path.insert(0, "/opt/trn_rl_repo")

from contextlib import ExitStack

import numpy as np

import concourse.bacc as bacc
import concourse.tile as tile
from concourse import mybir
from concourse.bass_utils import run_bass_kernel_spmd

F32R = mybir.dt.float32r
F32 = mybir.dt.float32
BF16 = mybir.dt.bfloat16
FP8 = mybir.dt.float8e4
DR = mybir.MatmulPerfMode.DoubleRow
AF = mybir.ActivationFunctionType
ALU = mybir.AluOpType

B = 2
S = 2048
M = 2048
D = 1024
H = 16
HD = 64
FF = 4 * D
NCORES = 8
SQ = 512          # own query tokens per rank
NB = S // 128     # 16 key/query blocks
DT = D // 128     # 8 feature tiles
FT = FF // 128    # 32 ffn feature tiles
DCA = H * (HD + 1)  # 1040 augmented V width
VG = 4            # V projection column groups (4 * 260)

# bias_all column map
BQ0, BK0, BO0, BQX0, BKX0, BOX0, B20 = 0, 8, 16, 24, 32, 40, 48
B10 = 56
LNG0 = 88
LNB0 = 112
EPS0 = 136
NBIAS = 137

_nc_cache = {}


def blocks_of(q):
    return sorted([q, 7 - q, 8 + q, 15 - q])


def _build():
    nc = bacc.Bacc(None, target_bir_lowering=False, num_devices=NCORES)
    dp = nc.declare_dram_parameter
    xT = dp("xT", [D, S], BF16, isOutput=False)
    xq = dp("xq", [D, SQ], BF16, isOutput=False)
    memT = dp("memT", [D, M], BF16, isOutput=False)
    maskT = dp("maskT", [128, NB * 128], BF16, isOutput=False)
    wq = dp("wq", [D, D], BF16, isOutput=False)
    wk = dp("wk", [D, D], BF16, isOutput=False)
    wv = dp("wv", [D, DCA], BF16, isOutput=False)
    wo = dp("wo", [D, D], BF16, isOutput=False)
    wqx = dp("wqx", [D, D], BF16, isOutput=False)
    wkx = dp("wkx", [D, D], BF16, isOutput=False)
    wvx = dp("wvx", [D, DCA], BF16, isOutput=False)
    wox = dp("wox", [D, D], BF16, isOutput=False)
    w1 = dp("w1", [D, FF], BF16, isOutput=False)
    w2 = dp("w2", [FF, D], BF16, isOutput=False)
    bva = dp("bva", [1, DCA], BF16, isOutput=False)
    bvxa = dp("bvxa", [1, DCA], BF16, isOutput=False)
    biasA = dp("biasA", [128, NBIAS], F32, isOutput=False)
    outT = dp("outT", [D, SQ], BF16, isOutput=True)

    with tile.TileContext(nc) as tc, ExitStack() as st:
        ep = st.enter_context
        constp = ep(tc.tile_pool(name="const", bufs=1))
        zp = ep(tc.tile_pool(name="zp", bufs=1))
        znp = ep(tc.tile_pool(name="znp", bufs=8))
        pp = ep(tc.tile_pool(name="pp", bufs=6))

        # ---- constants ----
        ba = constp.tile([128, NBIAS], F32, name="ba")
        nc.sync.dma_start(out=ba[:], in_=biasA[:, :])
        ones_t = constp.tile([128, 128], BF16, name="ones_t")
        nc.vector.memset(ones_t[:], 1.0 / D)
        bva_t = constp.tile([128, DCA], BF16, name="bva_t")
        nc.gpsimd.dma_start(out=bva_t[:], in_=bva[:, :].to_broadcast([128, DCA]))
        bvxa_t = constp.tile([128, DCA], BF16, name="bvxa_t")
        nc.gpsimd.dma_start(out=bvxa_t[:], in_=bvxa[:, :].to_broadcast([128, DCA]))

        def bcol(c):
            return ba[:, c:c + 1]

        # residual stream (bf16, persistent)
        z = [zp.tile([128, SQ], BF16, name=f"z{d}", tag=f"z{d}", bufs=1)
             for d in range(DT)]

        def layer_norm(src, dst, gcol, bcol_, mid_hook=None):
            """dst[d] = LN(src[d]); mid_hook emits filler PE work between
            the mean chain and the variance chain."""
            with tc.tile_pool(name="lnp", bufs=8) as lnp, \
                 tc.tile_pool(name="lns", bufs=1) as lns, \
                 tc.tile_pool(name="psLN", bufs=2, space="PSUM") as psLN:
                mu = psLN.tile([128, 512], F32, name="mu", tag="mu", bufs=1)
                for d in range(DT):
                    nc.tensor.matmul(out=mu[:], lhsT=ones_t[:], rhs=src[d][:],
                                     start=(d == 0), stop=(d == DT - 1))
                xm = [lnp.tile([128, SQ], BF16, name="xm", tag="xm", bufs=8)
                      for _ in range(DT)]
                for d in range(DT):
                    nc.vector.tensor_tensor(out=xm[d][:], in0=src[d][:],
                                            in1=mu[:], op=ALU.subtract)
                for d in range(DT):
                    sq = lnp.tile([128, 512], BF16, name="sq", tag="sq", bufs=8)
                    nc.scalar.activation(out=sq[:], in_=xm[d][:],
                                         func=AF.Square)
                    xm.append(sq)
                if mid_hook is not None:
                    mid_hook()
                var = psLN.tile([128, 512], F32, name="var", tag="var", bufs=1)
                for d in range(DT):
                    nc.tensor.matmul(out=var[:], lhsT=ones_t[:],
                                     rhs=xm[DT + d][:],
                                     start=(d == 0), stop=(d == DT - 1))
                std = lns.tile([128, 512], F32, name="std", tag="std", bufs=1)
                nc.scalar.activation(out=std[:], in_=var[:], func=AF.Sqrt,
                                     bias=bcol(EPS0), scale=1.0)
                rstd = lns.tile([128, 512], F32, name="rstd", tag="rstd",
                                bufs=1)
                nc.vector.reciprocal_approx_fast(rstd[:], std[:])
                for d in range(DT):
                    t = lnp.tile([128, 512], BF16, name="zt", tag="zt", bufs=2)
                    nc.vector.tensor_tensor(out=t[:], in0=xm[d][:],
                                            in1=rstd[:], op=ALU.mult)
                    nc.vector.tensor_scalar(
                        out=dst[d][:], in0=t[:], scalar1=bcol(gcol + d),
                        scalar2=bcol(bcol_ + d), op0=ALU.mult, op1=ALU.add)

        # ================= attention-scope pools =================
        with tc.tile_pool(name="kp", bufs=16) as kp, \
             tc.tile_pool(name="vp", bufs=16) as vp, \
             tc.tile_pool(name="qp", bufs=8) as qp, \
             tc.tile_pool(name="attnp", bufs=8) as attnp, \
             tc.tile_pool(name="dnp", bufs=2) as dnp:

            xq_stack = ExitStack()
            xqp = xq_stack.enter_context(tc.tile_pool(name="xqp", bufs=8))
            xq_t = []
            for d in range(DT):
                t = xqp.tile([128, SQ], BF16, name="xq", tag="xq", bufs=8)
                nc.gpsimd.dma_start(out=t[:], in_=xq[d * 128:(d + 1) * 128, :])
                xq_t.append(t)

            # ---- phase A: self K/V (x streamed per 512-token quarter), Q --
            k_t = [kp.tile([128, S], BF16, name="k", tag="k", bufs=16)
                   for _ in range(DT)]
            v_t = []
            with tc.tile_pool(name="xp", bufs=8) as xp, \
                 tc.tile_pool(name="wA", bufs=8) as wA, \
                 tc.tile_pool(name="wVA", bufs=8) as wVA, \
                 tc.tile_pool(name="psA", bufs=3, space="PSUM") as psA, \
                 tc.tile_pool(name="psV", bufs=3, space="PSUM") as psV:
                wk_t, wv_t = [], []
                WQS = [nc.scalar, nc.sync, nc.gpsimd]
                for d in range(DT):
                    t = wA.tile([128, D], BF16, name="wk", tag="wk", bufs=8)
                    WQS[d % 3].dma_start(out=t[:], in_=wk[d * 128:(d + 1) * 128, :])
                    wk_t.append(t)
                for d in range(DT):
                    t = wVA.tile([128, DCA], BF16, name="wv", tag="wv", bufs=8)
                    nc.sync.dma_start(out=t[:], in_=wv[d * 128:(d + 1) * 128, :])
                    wv_t.append(t)

                for ch in range(4):
                    csl = slice(ch * 512, (ch + 1) * 512)
                    x_t = []
                    XQS = [nc.sync, nc.gpsimd, nc.scalar, nc.sync]
                    for d in range(DT):
                        t = xp.tile([128, 512], BF16, name="x", tag="x",
                                    bufs=8)
                        XQS[d % 4].dma_start(
                            out=t[:], in_=xT[d * 128:(d + 1) * 128, csl])
                        x_t.append(t)
                    for t_i in range(DT):
                        ps = psA.tile([128, 512], F32, name="kps", tag="ps",
                                      bufs=3)
                        for d in range(DT):
                            nc.tensor.matmul(
                                out=ps[:],
                                lhsT=wk_t[d][:, t_i * 128:(t_i + 1) * 128],
                                rhs=x_t[d][:],
                                start=(d == 0), stop=(d == DT - 1))
                        nc.vector.tensor_scalar(
                            out=k_t[t_i][:, csl], in0=ps[:],
                            scalar1=bcol(BK0 + t_i), scalar2=None, op0=ALU.add)
                    for kb in range(4 * ch, 4 * ch + 4):
                        vt = vp.tile([128, DCA], BF16, name="v", tag="v",
                                     bufs=16)
                        lb = (kb - 4 * ch) * 128
                        for g in range(VG):
                            gsl = slice(g * 260, (g + 1) * 260)
                            ps = psV.tile([128, 260], F32, name="vps",
                                          tag="ps", bufs=3)
                            for d in range(DT):
                                nc.tensor.matmul(
                                    out=ps[:],
                                    lhsT=x_t[d][:, lb:lb + 128],
                                    rhs=wv_t[d][:, gsl],
                                    start=(d == 0), stop=(d == DT - 1))
                            nc.vector.tensor_tensor(
                                out=vt[:, gsl], in0=ps[:], in1=bva_t[:, gsl],
                                op=ALU.add)
                        v_t.append(vt)

                # self Q from xq (reuse wA ring for wq)
                wq_t = []
                for d in range(DT):
                    t = wA.tile([128, D], BF16, name="wq", tag="wk", bufs=8)
                    nc.scalar.dma_start(out=t[:], in_=wq[d * 128:(d + 1) * 128, :])
                    wq_t.append(t)
                q_t = []
                for t_i in range(DT):
                    qt = qp.tile([128, SQ], BF16, name="q", tag="q", bufs=8)
                    ps = psA.tile([128, 512], F32, name="qps", tag="ps",
                                  bufs=3)
                    for d in range(DT):
                        nc.tensor.matmul(
                            out=ps[:],
                            lhsT=wq_t[d][:, t_i * 128:(t_i + 1) * 128],
                            rhs=xq_t[d][:],
                            start=(d == 0), stop=(d == DT - 1))
                    nc.vector.tensor_scalar(
                        out=qt[:], in0=ps[:], scalar1=bcol(BQ0 + t_i),
                        scalar2=None, op0=ALU.add)
                    q_t.append(qt)

            # ---- shared attention emitter ----
            # Two heads in flight, PV delayed DD double-iters. With a causal
            # mask, each rank owns one query block per key-quartile, so for
            # key tile kb only the last (4 - kb//4) column blocks are live —
            # uniform across ranks. Scores/exp run on that suffix; only the
            # diagonal-quartile block needs the mask multiply.
            def attention(k_src, v_src, q_src, a_dst, mask_t, hook, psSC,
                          psPV, sc_bufs=3):
                DD = 2
                ND = NB // 2
                causal = mask_t is not None

                def wof(dkb):
                    return (4 - (2 * dkb) // 4) * 128 if causal else 512

                for hp_ in range(H // 2):
                    t_i = hp_
                    pvs = [psPV.tile([65, 512], F32, name="pv", tag="pv",
                                     bufs=2) for _ in range(2)]
                    pend = []
                    for dkb in range(ND + DD):
                        if dkb < ND:
                            W = wof(dkb)
                            c0 = 512 - W
                            pair = []
                            for par in range(2):
                                rows = slice(par * 64, par * 64 + 64)
                                sc = psSC.tile([128, 1024], F32, name="sc",
                                               tag="sc", bufs=3)
                                for j in range(2):
                                    kb = 2 * dkb + j
                                    nc.tensor.matmul(
                                        out=sc[:, j * W:(j + 1) * W],
                                        lhsT=k_src[t_i][rows,
                                                        kb * 128:(kb + 1) * 128],
                                        rhs=q_src[t_i][rows, c0:512],
                                        start=True, stop=True)
                                p = pp.tile([128, 1024], BF16, name="p",
                                            tag="p", bufs=6)
                                nc.scalar.activation(out=p[:, 0:2 * W],
                                                     in_=sc[:, 0:2 * W],
                                                     func=AF.Exp)
                                if causal:
                                    for j in range(2):
                                        kb = 2 * dkb + j
                                        nc.vector.tensor_tensor(
                                            out=p[:, j * W:j * W + 128],
                                            in0=p[:, j * W:j * W + 128],
                                            in1=mask_t[:, kb * 128:
                                                       (kb + 1) * 128],
                                            op=ALU.mult)
                                pair.append(p)
                            pend.append((dkb, pair))
                        if dkb >= DD:
                            d_, pair = pend.pop(0)
                            W = wof(d_)
                            c0 = 512 - W
                            for par in range(2):
                                h = 2 * hp_ + par
                                vsl = slice(h * 65, (h + 1) * 65)
                                for j in range(2):
                                    kb = 2 * d_ + j
                                    rp = pair[par]
                                    if causal and kb % 4 == 3:
                                        nc.tensor.matmul(
                                            out=pvs[par][:, c0:c0 + 128],
                                            lhsT=v_src[kb][:, vsl],
                                            rhs=rp[:, j * W:j * W + 128],
                                            start=(kb == 0), stop=True)
                                        if W > 128:
                                            nc.tensor.matmul(
                                                out=pvs[par][:, c0 + 128:512],
                                                lhsT=v_src[kb][:, vsl],
                                                rhs=rp[:, j * W + 128:
                                                       (j + 1) * W],
                                                start=False, stop=False)
                                    else:
                                        nc.tensor.matmul(
                                            out=pvs[par][:, c0:512],
                                            lhsT=v_src[kb][:, vsl],
                                            rhs=rp[:, j * W:(j + 1) * W],
                                            start=(kb == 0),
                                            stop=(not causal and kb == NB - 1))
                    for par in range(2):
                        h = 2 * hp_ + par
                        pv = pvs[par]
                        dsc = dnp.tile([65, 512], F32, name="dsc", tag="dsc",
                                       bufs=1)
                        nc.vector.tensor_scalar(
                            out=dsc[64:65, :], in0=pv[64:65, :], scalar1=1.0,
                            scalar2=None, op0=ALU.mult)
                        d0 = dnp.tile([1, 512], F32, name="d0", tag="d0",
                                      bufs=1)
                        nc.gpsimd.dma_start(out=d0[0:1, :], in_=dsc[64:65, :])
                        r0 = dnp.tile([1, 512], F32, name="r0", tag="r0",
                                      bufs=1)
                        nc.vector.reciprocal_approx_fast(r0[0:1, :],
                                                         d0[0:1, :])
                        rb0 = dnp.tile([1, 512], BF16, name="rb0", tag="rb0",
                                       bufs=1)
                        nc.vector.tensor_scalar(
                            out=rb0[0:1, :], in0=r0[0:1, :], scalar1=1.0,
                            scalar2=None, op0=ALU.mult)
                        rb = dnp.tile([64, 512], BF16, name="rb", tag="rb",
                                      bufs=2)
                        nc.gpsimd.partition_broadcast(rb[:], rb0[0:1, :])
                        if par == 0:
                            nc.vector.tensor_tensor(
                                out=a_dst[t_i][0:64, :], in0=pv[0:64, :],
                                in1=rb[:], op=ALU.mult)
                        else:
                            sh = dnp.tile([64, 512], BF16, name="sh", tag="sh",
                                          bufs=2)
                            nc.vector.tensor_tensor(out=sh[:], in0=pv[0:64, :],
                                                    in1=rb[:], op=ALU.mult)
                            nc.gpsimd.dma_start(out=a_dst[t_i][64:128, :],
                                                in_=sh[:])
                    if hook is not None:
                        hook(hp_)

            # ---- phase B: self-attn with cross-K interleaved ----
            kx_t = [kp.tile([128, M], BF16, name="kx", tag="k", bufs=16)
                    for _ in range(DT)]
            a_t = [attnp.tile([128, SQ], BF16, name="a", tag="a", bufs=8)
                   for _ in range(DT)]
            with tc.tile_pool(name="maskp", bufs=1) as maskp, \
                 tc.tile_pool(name="memp", bufs=8) as memp, \
                 tc.tile_pool(name="wB", bufs=8) as wB, \
                 tc.tile_pool(name="psSC", bufs=3, space="PSUM") as psSC, \
                 tc.tile_pool(name="psPV", bufs=2, space="PSUM") as psPV:
                mask_t = maskp.tile([128, NB * 128], BF16, name="mask")
                nc.sync.dma_start(out=mask_t[:], in_=maskT[:, :])
                wkx_t = []
                for d in range(DT):
                    t = wB.tile([128, D], BF16, name="wkx", tag="wkx", bufs=8)
                    nc.sync.dma_start(out=t[:], in_=wkx[d * 128:(d + 1) * 128, :])
                    wkx_t.append(t)
                mem_h = {}

                def load_mem_quarter(qv):
                    tiles = []
                    for d in range(DT):
                        t = memp.tile([128, 512], BF16, name="mem", tag="mem",
                                      bufs=8)
                        nc.sync.dma_start(
                            out=t[:],
                            in_=memT[d * 128:(d + 1) * 128,
                                     qv * 512:(qv + 1) * 512])
                        tiles.append(t)
                    mem_h[qv] = tiles

                load_mem_quarter(0)
                ck_items = [(qv, t_i) for qv in range(4) for t_i in range(DT)]

                def emit_cross_k():
                    qv, t_i = ck_items.pop(0)
                    if qv not in mem_h:
                        load_mem_quarter(qv)
                        mem_h.pop(qv - 2, None)
                    ps = psSC.tile([128, 1024], F32, name="ckps", tag="sc",
                                   bufs=3)
                    for d in range(DT):
                        nc.tensor.matmul(
                            out=ps[:, 0:512],
                            lhsT=wkx_t[d][:, t_i * 128:(t_i + 1) * 128],
                            rhs=mem_h[qv][d][:],
                            start=(d == 0), stop=(d == DT - 1))
                    nc.vector.tensor_scalar(
                        out=kx_t[t_i][:, qv * 512:(qv + 1) * 512],
                        in0=ps[:, 0:512], scalar1=bcol(BKX0 + t_i),
                        scalar2=None, op0=ALU.add)

                def hook_ck(h):
                    for _ in range(4):
                        if ck_items:
                            emit_cross_k()

                attention(k_t, v_t, q_t, a_t, mask_t, hook_ck, psSC, psPV)
                while ck_items:
                    emit_cross_k()

            # ---- phase C: self out-proj (t-outer, starts at first head) ----
            with tc.tile_pool(name="wO", bufs=8) as wO, \
                 tc.tile_pool(name="psO", bufs=8, space="PSUM") as psO:
                wo_t = []
                for t_i in range(DT):
                    t = wO.tile([128, D], BF16, name="wo", tag="wo", bufs=8)
                    (nc.sync if t_i % 2 else nc.gpsimd).dma_start(
                        out=t[:], in_=wo[t_i * 128:(t_i + 1) * 128, :])
                    wo_t.append(t)
                ps_o = [psO.tile([128, 512], F32, name=f"ops{d}", tag=f"ps{d}",
                                 bufs=1) for d in range(DT)]
                for t_i in range(DT):
                    for d in range(DT):
                        nc.tensor.matmul(
                            out=ps_o[d][:],
                            lhsT=wo_t[t_i][:, d * 128:(d + 1) * 128],
                            rhs=a_t[t_i][:],
                            start=(t_i == 0), stop=(t_i == DT - 1))
                for d in range(DT):
                    nc.vector.scalar_tensor_tensor(
                        out=z[d][:], in0=ps_o[d][:], scalar=bcol(BO0 + d),
                        in1=xq_t[d][:], op0=ALU.add, op1=ALU.add)
            xq_stack.close()

            # ---- phase D: cross V with LN1 interleaved, then cross Q ----
            zln = [znp.tile([128, SQ], BF16, name="zln", tag="zln", bufs=8)
                   for _ in range(DT)]
            vx_t = []
            with tc.tile_pool(name="memp2", bufs=8) as memp2, \
                 tc.tile_pool(name="wVX", bufs=8) as wVX, \
                 tc.tile_pool(name="psVX", bufs=3, space="PSUM") as psVX:
                wvx_t = []
                for d in range(DT):
                    t = wVX.tile([128, DCA], BF16, name="wvx", tag="wvx",
                                 bufs=8)
                    nc.sync.dma_start(out=t[:], in_=wvx[d * 128:(d + 1) * 128, :])
                    wvx_t.append(t)

                def crossv_qv(qv):
                    mem_t = []
                    for d in range(DT):
                        t = memp2.tile([128, 512], BF16, name="mem2",
                                       tag="mem", bufs=8)
                        nc.sync.dma_start(
                            out=t[:],
                            in_=memT[d * 128:(d + 1) * 128,
                                     qv * 512:(qv + 1) * 512])
                        mem_t.append(t)
                    for kb in range(4):
                        vt = vp.tile([128, DCA], BF16, name="vx", tag="v",
                                     bufs=16)
                        for g in range(VG):
                            gsl = slice(g * 260, (g + 1) * 260)
                            ps = psVX.tile([128, 260], F32, name="vxps",
                                           tag="ps", bufs=3)
                            for d in range(DT):
                                nc.tensor.matmul(
                                    out=ps[:],
                                    lhsT=mem_t[d][:, kb * 128:(kb + 1) * 128],
                                    rhs=wvx_t[d][:, gsl],
                                    start=(d == 0), stop=(d == DT - 1))
                            nc.vector.tensor_tensor(
                                out=vt[:, gsl], in0=ps[:], in1=bvxa_t[:, gsl],
                                op=ALU.add)
                        vx_t.append(vt)

                crossv_qv(0)
                crossv_qv(1)
                layer_norm(z, zln, LNG0, LNB0, mid_hook=lambda: crossv_qv(2))
                crossv_qv(3)

            qx_t = []
            with tc.tile_pool(name="wQX", bufs=8) as wQX, \
                 tc.tile_pool(name="psQX", bufs=2, space="PSUM") as psQX:
                wqx_t = []
                for d in range(DT):
                    t = wQX.tile([128, D], BF16, name="wqx", tag="wqx", bufs=8)
                    nc.sync.dma_start(out=t[:], in_=wqx[d * 128:(d + 1) * 128, :])
                    wqx_t.append(t)
                for t_i in range(DT):
                    qt = qp.tile([128, SQ], BF16, name="qx", tag="q", bufs=8)
                    ps = psQX.tile([128, 512], F32, name="qxps", tag="ps",
                                   bufs=2)
                    for d in range(DT):
                        nc.tensor.matmul(
                            out=ps[:],
                            lhsT=wqx_t[d][:, t_i * 128:(t_i + 1) * 128],
                            rhs=zln[d][:],
                            start=(d == 0), stop=(d == DT - 1))
                    nc.vector.tensor_scalar(
                        out=qt[:], in0=ps[:], scalar1=bcol(BQX0 + t_i),
                        scalar2=None, op0=ALU.add)
                    qx_t.append(qt)

            # ---- phase E: cross attention; out-proj; LN2 ----
            ax_t = [attnp.tile([128, SQ], BF16, name="ax", tag="a", bufs=8)
                    for _ in range(DT)]
            with tc.tile_pool(name="wOX", bufs=8) as wOX:
                wox_t = []
                for t_i in range(DT):
                    t = wOX.tile([128, D], BF16, name="wox", tag="wox", bufs=8)
                    nc.sync.dma_start(out=t[:], in_=wox[t_i * 128:(t_i + 1) * 128, :])
                    wox_t.append(t)
                with tc.tile_pool(name="psSC2", bufs=3, space="PSUM") as psSC2, \
                     tc.tile_pool(name="psPV2", bufs=2, space="PSUM") as psPV2:
                    attention(kx_t, vx_t, qx_t, ax_t, None, None, psSC2,
                              psPV2)
                with tc.tile_pool(name="psOX", bufs=8, space="PSUM") as psOX:
                    ps_x = [psOX.tile([128, 512], F32, name=f"oxps{d}",
                                      tag=f"ps{d}", bufs=1)
                            for d in range(DT)]
                    for t_i in range(DT):
                        for d in range(DT):
                            nc.tensor.matmul(
                                out=ps_x[d][:],
                                lhsT=wox_t[t_i][:, d * 128:(d + 1) * 128],
                                rhs=ax_t[t_i][:],
                                start=(t_i == 0), stop=(t_i == DT - 1))
                    for d in range(DT):
                        nc.vector.scalar_tensor_tensor(
                            out=z[d][:], in0=ps_x[d][:], scalar=bcol(BOX0 + d),
                            in1=zln[d][:], op0=ALU.add, op1=ALU.add)

        # attention pools closed — FFN has the SBUF to itself
        zln2 = [znp.tile([128, SQ], BF16, name="zln2", tag="zln", bufs=8)
                for _ in range(DT)]
        layer_norm(z, zln2, LNG0 + DT, LNB0 + DT)

        with tc.tile_pool(name="hp", bufs=32) as hp, \
             tc.tile_pool(name="w1p", bufs=8) as w1p, \
             tc.tile_pool(name="w2p", bufs=4) as w2p:
            with tc.tile_pool(name="psF1", bufs=3, space="PSUM") as psF1:
                h_t = []
                w1_t = {}
                for fh in range(2):
                    for d in range(DT):
                        t = w1p.tile([128, 2048], BF16, name="w1", tag="w1",
                                     bufs=8)
                        (nc.sync if d % 2 else nc.gpsimd).dma_start(
                            out=t[:],
                            in_=w1[d * 128:(d + 1) * 128,
                                   fh * 2048:(fh + 1) * 2048])
                        w1_t[(fh, d)] = t
                    for f in range(fh * 16, fh * 16 + 16):
                        fo = f * 128 - fh * 2048
                        ht = hp.tile([128, SQ], BF16, name="h", tag="h",
                                     bufs=32)
                        ps = psF1.tile([128, 512], F32, name="f1ps", tag="ps",
                                       bufs=3)
                        for d in range(DT):
                            nc.tensor.matmul(
                                out=ps[:],
                                lhsT=w1_t[(fh, d)][:, fo:fo + 128],
                                rhs=zln2[d][:],
                                start=(d == 0), stop=(d == DT - 1))
                        nc.scalar.activation(out=ht[:], in_=ps[:],
                                             func=AF.Relu,
                                             bias=bcol(B10 + f), scale=1.0)
                        h_t.append(ht)
            with tc.tile_pool(name="psF2", bufs=8, space="PSUM") as psF2:
                ps_d = [psF2.tile([128, 512], F32, name=f"f2ps{d}",
                                  tag=f"ps{d}", bufs=1) for d in range(DT)]
                for f in range(FT):
                    w2t = w2p.tile([128, D], BF16, name="w2", tag="w2", bufs=4)
                    (nc.sync if f % 2 else nc.gpsimd).dma_start(
                        out=w2t[:], in_=w2[f * 128:(f + 1) * 128, :])
                    for d in range(DT):
                        nc.tensor.matmul(
                            out=ps_d[d][:],
                            lhsT=w2t[:, d * 128:(d + 1) * 128],
                            rhs=h_t[f][:],
                            start=(f == 0), stop=(f == FT - 1))
                for d in range(DT):
                    nc.vector.scalar_tensor_tensor(
                        out=z[d][:], in0=ps_d[d][:], scalar=bcol(B20 + d),
                        in1=zln2[d][:], op0=ALU.add, op1=ALU.add)

        zout = [znp.tile([128, SQ], BF16, name="zo", tag="zln", bufs=8)
                for _ in range(DT)]
        layer_norm(z, zout, LNG0 + 2 * DT, LNB0 + 2 * DT)
        for d in range(DT):
            nc.sync.dma_start(out=outT[d * 128:(d + 1) * 128, :],
                              in_=zout[d][:])

    nc.finalize()
    return nc


def _get_nc():
    if "nc" not in _nc_cache:
        _nc_cache["nc"] = _build()
    return _nc_cache["nc"]


def _aug_v(wv_c, bv_c, bf16):
    wva = np.zeros((D, DCA), np.float32)
    bva = np.zeros((1, DCA), np.float32)
    for h in range(H):
        wva[:, h * 65:h * 65 + 64] = wv_c[:, h * 64:(h + 1) * 64]
        bva[0, h * 65:h * 65 + 64] = bv_c[h * 64:(h + 1) * 64]
        bva[0, h * 65 + 64] = 1.0
    return wva.astype(bf16), bva.astype(bf16)


def _prep_shared(tgt, memory, tgt_mask, Wqkv, bqkv, Wo_sa, bo_sa, Wq, bq, Wk,
                 bk, Wv, bv, Wo_cx, bo_cx, W1, b1, W2, b2, g_mha, bn_mha,
                 g_crx, bn_crx, g_ffn, bn_ffn):
    import ml_dtypes
    bf16 = ml_dtypes.bfloat16
    fp8 = ml_dtypes.float8_e4m3fn
    scale = np.float32(1.0 / np.sqrt(HD))
    wqkv_h = Wqkv.reshape(D, H, 3 * HD)
    bqkv_h = bqkv.reshape(H, 3 * HD)
    wq_sa = (wqkv_h[:, :, 0:HD].reshape(D, D) * scale).astype(bf16)
    wk_sa = wqkv_h[:, :, HD:2 * HD].reshape(D, D).astype(bf16)
    wv_sa, bva_sa = _aug_v(wqkv_h[:, :, 2 * HD:].reshape(D, D),
                           bqkv_h[:, 2 * HD:].reshape(D), bf16)
    wvx_a, bvx_a = _aug_v(Wv, bv, bf16)
    bias = np.zeros((128, NBIAS), np.float32)

    def put(col0, vec):
        n = vec.shape[0] // 128
        bias[:, col0:col0 + n] = vec.reshape(n, 128).T

    put(BQ0, bqkv_h[:, 0:HD].reshape(D) * scale)
    put(BK0, bqkv_h[:, HD:2 * HD].reshape(D))
    put(BO0, bo_sa)
    put(BQX0, bq * scale)
    put(BKX0, bk)
    put(BOX0, bo_cx)
    put(B20, b2)
    put(B10, b1)
    put(LNG0, np.concatenate([g_mha, g_crx, g_ffn]))
    put(LNB0, np.concatenate([bn_mha, bn_crx, bn_ffn]))
    bias[:, EPS0] = 1e-5

    w = {
        "wq": np.ascontiguousarray(wq_sa),
        "wk": np.ascontiguousarray(wk_sa),
        "wv": np.ascontiguousarray(wv_sa),
        "wo": Wo_sa.astype(bf16),
        "wqx": (Wq * scale).astype(bf16),
        "wkx": Wk.astype(bf16),
        "wvx": np.ascontiguousarray(wvx_a),
        "wox": Wo_cx.astype(bf16),
        "w1": W1.astype(bf16),
        "w2": W2.astype(bf16),
        "bva": bva_sa.reshape(1, DCA),
        "bvxa": bvx_a.reshape(1, DCA),
        "biasA": bias,
    }
    xTs = [np.ascontiguousarray(tgt[i].T).astype(bf16) for i in range(B)]
    memTs = [np.ascontiguousarray(memory[i].T).astype(bf16) for i in range(B)]
    masks = {}
    kpos = np.arange(128)
    for q in range(4):
        blks = blocks_of(q)
        m = np.zeros((128, NB * 128), np.float32)
        for kb in range(NB):
            b = blks[kb // 4]  # this rank's block in kb's quartile
            qq = b * 128 + np.arange(128)
            kk = kb * 128 + kpos
            m[:, kb * 128:(kb + 1) * 128] = ~tgt_mask[qq[None, :], kk[:, None]]
        masks[q] = m.astype(bf16)
    return w, xTs, memTs, masks


def kernel(**inputs):
    tgt = np.asarray(inputs["tgt"], np.float32)
    memory = np.asarray(inputs["memory"], np.float32)
    mask = np.asarray(inputs["tgt_mask"], bool)
    args = {k: np.asarray(v, np.float32) for k, v in inputs.items()
            if k not in ("tgt", "memory", "tgt_mask")}

    nc = _get_nc()
    w, xTs, memTs, masks = _prep_shared(tgt, memory, mask, **args)
    in_maps = []
    for r in range(NCORES):
        b, q = r // 4, r % 4
        cols = np.concatenate([np.arange(bk_ * 128, (bk_ + 1) * 128)
                               for bk_ in blocks_of(q)])
        im = dict(w)
        im["xT"] = xTs[b]
        im["xq"] = np.ascontiguousarray(xTs[b][:, cols])
        im["memT"] = memTs[b]
        im["maskT"] = masks[q]
        in_maps.append(im)

    trace = bool(int(os.environ.get("BASS_KERNEL_TRACE", "0")))
    res = run_bass_kernel_spmd(nc, in_maps, list(range(NCORES)), trace=trace)
    if trace:
        kernel.last_exec_time_ns = res.exec_time_ns

    out = np.empty((B, S, D), np.float32)
    for r in range(NCORES):
        b, q = r // 4, r % 4
        cols = np.concatenate([np.arange(bk_ * 128, (bk_ + 1) * 128)
                               for bk_ in blocks_of(q)])
        out[b, cols, :] = res.results[r]["outT"].astype(np.float32).T
    return out
